# revision 1
# baseline (speedup 1.0000x reference)
"""CrossNetwork (4-layer DCN-v1) Trainium2 Bass kernel.

Math: the reference computes, with x0 = x:
    x_{i+1} = x0 * <x_i, w_i> + b_i + x_i          (i = 0..3)

Every x_i stays of the form  x_i = a_i[row] * x0 + c_i  with
    c_i = sum_{j<i} b_j                (row independent)
    a_{i+1} = a_i * (1 + d_i) + e_i    (per-row scalar recurrence)
    d_i = <x0_row, w_i>                (per-row dots, vs x0 only)
    e_i = <c_i, w_i>                   (scalar constants)
so the full network collapses to:
    out = a_4[:, None] * x0 + (b_0+b_1+b_2+b_3)[None, :]

On-chip per 512-row supertile (layout [128 part, 4 r, 1024 f], row = 4p+r):
  1. PE transposes x chunks (regular matmul vs identity) -> xT in PSUM
  2. ACT copies xT PSUM->SBUF
  3. PE dot-matmuls  D^T[4, 512] += Wc^T @ xTc   (accumulate over 8 f-chunks)
  4. PE transposes D^T -> D[128, 4r, 4i], ACT copies to SBUF
  5. DVE: 1+D, then tensor_tensor_scan implements the a-recurrence
  6. DVE scalar_tensor_tensor: out = (x * a) + csum_rep  (single pass)
Sharding: batch dim across 8 cores (4096 rows each), SPMD.
"""

import numpy as np

import concourse.bacc as bacc
import concourse.bass as bass
import concourse.mybir as mybir
import concourse.tile as tile
from concourse.bass_utils import run_bass_kernel_spmd
from concourse.masks import make_identity

N_CORES = 8
B, F, L = 32768, 1024, 4
BS = B // N_CORES          # 4096 rows per core
ST_ROWS = 512              # rows per supertile
N_ST = BS // ST_ROWS       # 8 supertiles per core
R = ST_ROWS // 128         # 4 row-combs per supertile
NCHUNK = F // 128          # 8 feature chunks

FP32 = mybir.dt.float32
ADD = mybir.AluOpType.add
MULT = mybir.AluOpType.mult

_PROGRAM_CACHE = {}


def _build_program(transpose_mode=True, dot_f32r=False, transpose_f32r=False):
    F32R = mybir.dt.float32r
    nc = bacc.Bacc("TRN2")
    x_d = nc.dram_tensor("x", [BS, F], FP32, kind="ExternalInput")
    w_d = nc.dram_tensor("wpack", [128, NCHUNK, L], FP32, kind="ExternalInput")
    e_d = nc.dram_tensor("erep", [128, L], FP32, kind="ExternalInput")
    c_d = nc.dram_tensor("crep", [128, F], FP32, kind="ExternalInput")
    o_d = nc.dram_tensor("out", [BS, F], FP32, kind="ExternalOutput")

    with tile.TileContext(nc) as tc:
        with (
            tc.tile_pool(name="const", bufs=1) as const_pool,
            tc.tile_pool(name="xin", bufs=4) as xpool,
            tc.tile_pool(name="oout", bufs=4) as opool,
            tc.tile_pool(name="xtsb", bufs=3) as xtpool,
            tc.tile_pool(name="small", bufs=2) as spool,
            tc.tile_pool(name="xtps", bufs=3, space="PSUM") as tpsum,
            tc.tile_pool(name="dtps", bufs=2, space="PSUM") as dpsum,
            tc.tile_pool(name="dps", bufs=2, space="PSUM") as dpsum2,
        ):
            ident = const_pool.tile([128, 128], FP32)
            make_identity(nc, ident[:])
            wsb = const_pool.tile([128, NCHUNK, L], FP32)
            nc.sync.dma_start(out=wsb[:], in_=w_d[:])
            esb = const_pool.tile([128, L], FP32)
            nc.sync.dma_start(out=esb[:], in_=e_d[:])
            csb = const_pool.tile([128, F], FP32)
            nc.sync.dma_start(out=csb[:], in_=c_d[:])

            for st in range(N_ST):
                x_t = xpool.tile([128, R, F], FP32)
                src = x_d[st * ST_ROWS:(st + 1) * ST_ROWS, :].rearrange(
                    "(p r) f -> p r f", p=128)
                nc.sync.dma_start(out=x_t[:], in_=src)

                # D^T[i, r*128+j] accumulates sum_f w[i,f] * x[row(4j+r), f]
                dt_ps = dpsum.tile([L, R * 128], FP32)
                for c in range(NCHUNK):
                    xt_ps = tpsum.tile([128, R * 128], FP32)
                    for r in range(R):
                        # out = x_chunk^T (PE transpose path or regular matmul
                        # against identity)
                        src_ap = x_t[:, r, c * 128:(c + 1) * 128]
                        id_ap = ident[:]
                        if transpose_f32r:
                            src_ap = src_ap.bitcast(F32R)
                            id_ap = id_ap.bitcast(F32R)
                        nc.tensor.matmul(
                            xt_ps[:, r * 128:(r + 1) * 128],
                            src_ap,
                            id_ap,
                            start=True, stop=True,
                            is_transpose=transpose_mode or None,
                        )
                    xt_sb = xtpool.tile([128, R * 128], FP32)
                    nc.scalar.copy(xt_sb[:], xt_ps[:])
                    w_ap = wsb[:, c, :]
                    xt_ap = xt_sb[:]
                    if dot_f32r:
                        w_ap = w_ap.bitcast(F32R)
                        xt_ap = xt_ap.bitcast(F32R)
                    nc.tensor.matmul(
                        dt_ps[:],
                        w_ap,
                        xt_ap,
                        start=(c == 0), stop=(c == NCHUNK - 1),
                    )

                dt_sb = spool.tile([L, R * 128], FP32, tag="dt_sb")
                nc.scalar.copy(dt_sb[:], dt_ps[:])

                # transpose D^T -> D [128 j, r, i]
                d_ps = dpsum2.tile([128, R, L], FP32)
                for r in range(R):
                    nc.tensor.matmul(
                        d_ps[:, r, :],
                        dt_sb[:, r * 128:(r + 1) * 128],
                        ident[:L, :L],
                        start=True, stop=True,
                    )
                d_sb = spool.tile([128, R, L], FP32, tag="d_sb")
                nc.scalar.copy(d_sb[:], d_ps[:])

                # a-recurrence: state=1; state = ((1+d_i)*state) + e_i
                pd = spool.tile([128, R, L], FP32, tag="pd")
                nc.vector.tensor_scalar_add(pd[:], d_sb[:], 1.0)
                sc = spool.tile([128, R, L], FP32, tag="sc")
                for r in range(R):
                    nc.vector.tensor_tensor_scan(
                        sc[:, r, :], pd[:, r, :], esb[:],
                        1.0, MULT, ADD,
                    )

                # epilogue: out = (x * a) + csum  in one DVE pass per comb
                o_t = opool.tile([128, R, F], FP32)
                for r in range(R):
                    nc.vector.scalar_tensor_tensor(
                        o_t[:, r, :], x_t[:, r, :], sc[:, r, L - 1:L], csb[:],
                        MULT, ADD,
                    )
                dst = o_d[st * ST_ROWS:(st + 1) * ST_ROWS, :].rearrange(
                    "(p r) f -> p r f", p=128)
                # stores go out on the (otherwise idle) GpSimd SWDGE queue so
                # they never head-of-line block the next supertile's load on
                # the sync HWDGE queue
                nc.gpsimd.dma_start(out=dst, in_=o_t[:])
    nc.compile()
    return nc


def _host_prep(Ws, Bs):
    Ws = np.asarray(Ws, dtype=np.float32)
    Bs = np.asarray(Bs, dtype=np.float32)
    # wpack[p, c, i] = Ws[i, c*128 + p]
    wpack = np.ascontiguousarray(
        Ws.reshape(L, NCHUNK, 128).transpose(2, 1, 0))
    csum = np.zeros(F, np.float32)
    e = np.zeros(L, np.float32)
    for i in range(L):
        e[i] = np.float32(csum @ Ws[i])
        csum = (csum + Bs[i]).astype(np.float32)
    erep = np.broadcast_to(e, (128, L)).copy()
    crep = np.broadcast_to(csum, (128, F)).copy()
    return wpack, erep, crep


def _get_program(**opts):
    key = tuple(sorted(opts.items()))
    if key not in _PROGRAM_CACHE:
        _PROGRAM_CACHE[key] = _build_program(**opts)
    return _PROGRAM_CACHE[key]


def _in_maps(x, Ws, Bs):
    x = np.asarray(x, dtype=np.float32)
    wpack, erep, crep = _host_prep(Ws, Bs)
    return [
        {
            "x": np.ascontiguousarray(x[k * BS:(k + 1) * BS]),
            "wpack": wpack,
            "erep": erep,
            "crep": crep,
        }
        for k in range(N_CORES)
    ]


def _run(x, Ws, Bs, trace=False, trace_kwargs=None, **opts):
    nc = _get_program(**opts)
    in_maps = _in_maps(x, Ws, Bs)
    res = run_bass_kernel_spmd(
        nc, in_maps, list(range(N_CORES)),
        trace=trace, **(trace_kwargs or {}),
    )
    out = np.concatenate([res.results[k]["out"] for k in range(N_CORES)], axis=0)
    return out, res


def kernel(x, Ws, Bs):
    out, _ = _run(x, Ws, Bs, trace=False)
    return out



# revision 4
# speedup vs baseline: 1.0119x; 1.0119x over previous
"""CrossNetwork (4-layer DCN-v1) Trainium2 Bass kernel.

Math: the reference computes, with x0 = x:
    x_{i+1} = x0 * <x_i, w_i> + b_i + x_i          (i = 0..3)

Every x_i stays of the form  x_i = a_i[row] * x0 + c_i  with
    c_i = sum_{j<i} b_j                (row independent)
    a_{i+1} = a_i * (1 + d_i) + e_i    (per-row scalar recurrence)
    d_i = <x0_row, w_i>                (per-row dots, vs x0 only)
    e_i = <c_i, w_i>                   (scalar constants)
so the full network collapses to:
    out = a_4[:, None] * x0 + (b_0+b_1+b_2+b_3)[None, :]

On-chip per 512-row supertile (layout [128 part, 4 r, 1024 f], row = 4p+r):
  1. PE transposes x chunks -> xT in PSUM
  2. ACT copies xT PSUM->SBUF
  3. PE dot-matmuls  D^T[4, 512] += Wc^T @ xTc   (accumulate over 8 f-chunks)
  4. PE transposes D^T -> D[128, 4r, 4i], ACT copies to SBUF
  5. DVE: 1+D, then tensor_tensor_scan implements the a-recurrence
  6. DVE scalar_tensor_tensor: x_t = (x_t * a) + csum_rep, in place
  7. store x_t (now holding the output) on the SWDGE queue

All 8 supertiles stay resident in SBUF (16KB/partition each), so the 8
input loads pack back-to-back on the sync HWDGE queue while stores
drain on the (otherwise idle) GpSimd SWDGE queue -- DMA, the roofline
resource, never waits on a buffer recycle. Const loads also go out on
the SWDGE queue so the first x load starts immediately.
Sharding: batch dim across 8 cores (4096 rows each), SPMD.
"""

import numpy as np

import concourse.bacc as bacc
import concourse.bass as bass
import concourse.mybir as mybir
import concourse.tile as tile
from concourse.bass_utils import run_bass_kernel_spmd
from concourse.masks import make_identity

N_CORES = 8
B, F, L = 32768, 1024, 4
BS = B // N_CORES          # 4096 rows per core
ST_ROWS = 512              # rows per supertile
N_ST = BS // ST_ROWS       # 8 supertiles per core
R = ST_ROWS // 128         # 4 row-combs per supertile
NCHUNK = F // 128          # 8 feature chunks

FP32 = mybir.dt.float32
ADD = mybir.AluOpType.add
MULT = mybir.AluOpType.mult

_PROGRAM_CACHE = {}


def _build_program(dot_f32r=True, transpose_f32r=False, outer_iters=1):
    F32R = mybir.dt.float32r
    nc = bacc.Bacc("TRN2")
    x_d = nc.dram_tensor("x", [BS, F], FP32, kind="ExternalInput")
    w_d = nc.dram_tensor("wpack", [128, NCHUNK, L], FP32, kind="ExternalInput")
    e_d = nc.dram_tensor("erep", [128, L], FP32, kind="ExternalInput")
    c_d = nc.dram_tensor("crep", [128, F], FP32, kind="ExternalInput")
    o_d = nc.dram_tensor("out", [BS, F], FP32, kind="ExternalOutput")

    with tile.TileContext(nc) as tc:
        with (
            tc.tile_pool(name="const", bufs=1) as const_pool,
            tc.tile_pool(name="xin", bufs=N_ST) as xpool,
            tc.tile_pool(name="xtsb", bufs=3) as xtpool,
            tc.tile_pool(name="small", bufs=2) as spool,
            tc.tile_pool(name="xtps", bufs=3, space="PSUM") as tpsum,
            tc.tile_pool(name="dtps", bufs=2, space="PSUM") as dpsum,
            tc.tile_pool(name="dps", bufs=2, space="PSUM") as dpsum2,
        ):
            ident = const_pool.tile([128, 128], FP32)
            make_identity(nc, ident[:])
            # consts ride the SWDGE queue: the sync HWDGE queue then opens
            # with the first x load, not 1.6us of w/e/c traffic
            wsb_raw = const_pool.tile([128, NCHUNK, L], FP32)
            nc.gpsimd.dma_start(out=wsb_raw[:], in_=w_d[:])
            if dot_f32r:
                # f32r matmul operands must be produced pre-rounded: convert
                # once via ACT (the per-chunk xT copies below do the same)
                wsb = const_pool.tile([128, NCHUNK, L], F32R)
                nc.scalar.copy(wsb[:], wsb_raw[:])
            else:
                wsb = wsb_raw
            esb = const_pool.tile([128, L], FP32)
            nc.gpsimd.dma_start(out=esb[:], in_=e_d[:])
            csb = const_pool.tile([128, F], FP32)
            nc.gpsimd.dma_start(out=csb[:], in_=c_d[:])

            def body():
                for st in range(N_ST):
                    x_t = xpool.tile([128, R, F], FP32)
                    src = x_d[st * ST_ROWS:(st + 1) * ST_ROWS, :].rearrange(
                        "(p r) f -> p r f", p=128)
                    nc.sync.dma_start(out=x_t[:], in_=src)

                    # D^T[i, r*128+j] += sum_f w[i,f] * x[row(4j+r), f]
                    dt_ps = dpsum.tile([L, R * 128], FP32)
                    for c in range(NCHUNK):
                        xt_ps = tpsum.tile([128, R * 128], FP32)
                        for r in range(R):
                            src_ap = x_t[:, r, c * 128:(c + 1) * 128]
                            id_ap = ident[:]
                            if transpose_f32r:
                                src_ap = src_ap.bitcast(F32R)
                                id_ap = id_ap.bitcast(F32R)
                            nc.tensor.matmul(
                                xt_ps[:, r * 128:(r + 1) * 128],
                                src_ap,
                                id_ap,
                                start=True, stop=True,
                                is_transpose=True,
                            )
                        xt_sb = xtpool.tile(
                            [128, R * 128], F32R if dot_f32r else FP32)
                        nc.scalar.copy(xt_sb[:], xt_ps[:])
                        nc.tensor.matmul(
                            dt_ps[:],
                            wsb[:, c, :],
                            xt_sb[:],
                            start=(c == 0), stop=(c == NCHUNK - 1),
                        )

                    dt_sb = spool.tile([L, R * 128], FP32, tag="dt_sb")
                    nc.scalar.copy(dt_sb[:], dt_ps[:])

                    # transpose D^T -> D [128 j, r, i]
                    d_ps = dpsum2.tile([128, R, L], FP32)
                    for r in range(R):
                        nc.tensor.matmul(
                            d_ps[:, r, :],
                            dt_sb[:, r * 128:(r + 1) * 128],
                            ident[:L, :L],
                            start=True, stop=True,
                        )
                    d_sb = spool.tile([128, R, L], FP32, tag="d_sb")
                    nc.scalar.copy(d_sb[:], d_ps[:])

                    # a-recurrence: state=1; state = ((1+d_i)*state) + e_i
                    pd = spool.tile([128, R, L], FP32, tag="pd")
                    nc.vector.tensor_scalar_add(pd[:], d_sb[:], 1.0)
                    sc = spool.tile([128, R, L], FP32, tag="sc")
                    for r in range(R):
                        nc.vector.tensor_tensor_scan(
                            sc[:, r, :], pd[:, r, :], esb[:],
                            1.0, MULT, ADD,
                        )

                    # epilogue in place: x_t = (x_t * a) + csum
                    for r in range(R):
                        nc.vector.scalar_tensor_tensor(
                            x_t[:, r, :], x_t[:, r, :], sc[:, r, L - 1:L],
                            csb[:], MULT, ADD,
                        )
                    dst = o_d[st * ST_ROWS:(st + 1) * ST_ROWS, :].rearrange(
                        "(p r) f -> p r f", p=128)
                    # stores on the SWDGE queue so they never head-of-line
                    # block the next load on the sync HWDGE queue
                    nc.gpsimd.dma_start(out=dst, in_=x_t[:])

            if outer_iters == 1:
                body()
            else:
                with tc.For_i(0, outer_iters):
                    body()
    nc.compile()
    return nc


def _host_prep(Ws, Bs):
    Ws = np.asarray(Ws, dtype=np.float32)
    Bs = np.asarray(Bs, dtype=np.float32)
    # wpack[p, c, i] = Ws[i, c*128 + p]
    wpack = np.ascontiguousarray(
        Ws.reshape(L, NCHUNK, 128).transpose(2, 1, 0))
    csum = np.zeros(F, np.float32)
    e = np.zeros(L, np.float32)
    for i in range(L):
        e[i] = np.float32(csum @ Ws[i])
        csum = (csum + Bs[i]).astype(np.float32)
    erep = np.broadcast_to(e, (128, L)).copy()
    crep = np.broadcast_to(csum, (128, F)).copy()
    return wpack, erep, crep


def _get_program(**opts):
    key = tuple(sorted(opts.items()))
    if key not in _PROGRAM_CACHE:
        _PROGRAM_CACHE[key] = _build_program(**opts)
    return _PROGRAM_CACHE[key]


def _in_maps(x, Ws, Bs):
    x = np.asarray(x, dtype=np.float32)
    wpack, erep, crep = _host_prep(Ws, Bs)
    return [
        {
            "x": np.ascontiguousarray(x[k * BS:(k + 1) * BS]),
            "wpack": wpack,
            "erep": erep,
            "crep": crep,
        }
        for k in range(N_CORES)
    ]


def _run(x, Ws, Bs, trace=False, trace_kwargs=None, **opts):
    nc = _get_program(**opts)
    in_maps = _in_maps(x, Ws, Bs)
    res = run_bass_kernel_spmd(
        nc, in_maps, list(range(N_CORES)),
        trace=trace, **(trace_kwargs or {}),
    )
    out = np.concatenate([res.results[k]["out"] for k in range(N_CORES)], axis=0)
    return out, res


def kernel(x, Ws, Bs):
    out, _ = _run(x, Ws, Bs, trace=False)
    return out


# revision 16
# speedup vs baseline: 1.0204x; 1.0084x over previous
"""CrossNetwork (4-layer DCN-v1) Trainium2 Bass kernel.

Math: the reference computes, with x0 = x:
    x_{i+1} = x0 * <x_i, w_i> + b_i + x_i          (i = 0..3)

Every x_i stays of the form  x_i = a_i[row] * x0 + c_i  with
    c_i = sum_{j<i} b_j                (row independent)
    a_{i+1} = a_i * (1 + d_i) + e_i    (per-row scalar recurrence)
    d_i = <x0_row, w_i>                (per-row dots, vs x0 only)
    e_i = <c_i, w_i>                   (scalar constants)
so the full network collapses to:
    out = a_4[:, None] * x0 + (b_0+b_1+b_2+b_3)[None, :]

On-chip per 512-row supertile (layout [128 part, 4 r, 1024 f], row = 4p+r):
  1. PE transposes x chunks -> xT in PSUM
  2. ACT copies xT PSUM->SBUF
  3. PE dot-matmuls  D^T[4, 512] += Wc^T @ xTc   (accumulate over 8 f-chunks)
  4. PE transposes D^T -> D[128, 4r, 4i], ACT copies to SBUF
  5. DVE: 1+D, then tensor_tensor_scan implements the a-recurrence
  6. DVE scalar_tensor_tensor: x_t = (x_t * a) + csum_rep, in place
  7. store x_t (now holding the output) on the SWDGE queue

All 8 supertiles stay resident in SBUF (16KB/partition each), so the 8
input loads pack back-to-back on the sync HWDGE queue while stores
drain on the (otherwise idle) GpSimd SWDGE queue -- DMA, the roofline
resource, never waits on a buffer recycle. Const loads also go out on
the SWDGE queue so the first x load starts immediately.
Sharding: batch dim across 8 cores (4096 rows each), SPMD.
"""

import numpy as np

import concourse.bacc as bacc
import concourse.bass as bass
import concourse.mybir as mybir
import concourse.tile as tile
from concourse.bass_utils import run_bass_kernel_spmd
from concourse.masks import make_identity

N_CORES = 8
B, F, L = 32768, 1024, 4
BS = B // N_CORES          # 4096 rows per core
ST_ROWS = 512              # rows per supertile
N_ST = BS // ST_ROWS       # 8 supertiles per core
R = ST_ROWS // 128         # 4 row-combs per supertile
NCHUNK = F // 128          # 8 feature chunks

FP32 = mybir.dt.float32
ADD = mybir.AluOpType.add
MULT = mybir.AluOpType.mult

_PROGRAM_CACHE = {}


def _build_program(dot_f32r=True, transpose_f32r=False, outer_iters=1):
    F32R = mybir.dt.float32r
    nc = bacc.Bacc("TRN2")
    x_d = nc.dram_tensor("x", [BS, F], FP32, kind="ExternalInput")
    w_d = nc.dram_tensor("wpack", [128, NCHUNK, L], FP32, kind="ExternalInput")
    e_d = nc.dram_tensor("erep", [128, L], FP32, kind="ExternalInput")
    c_d = nc.dram_tensor("crow", [1, F], FP32, kind="ExternalInput")
    o_d = nc.dram_tensor("out", [BS, F], FP32, kind="ExternalOutput")

    with tile.TileContext(nc) as tc:
        with (
            tc.tile_pool(name="const", bufs=1) as const_pool,
            tc.tile_pool(name="xin", bufs=N_ST) as xpool,
            tc.tile_pool(name="xtsb", bufs=3) as xtpool,
            tc.tile_pool(name="small", bufs=2) as spool,
            tc.tile_pool(name="xtps", bufs=3, space="PSUM") as tpsum,
            tc.tile_pool(name="dtps", bufs=2, space="PSUM") as dpsum,
            tc.tile_pool(name="dps", bufs=2, space="PSUM") as dpsum2,
        ):
            # supertile 0's load is issued before anything else on the sync
            # queue so the DMA head starts as early as possible
            x0_t = xpool.tile([128, R, F], FP32, tag="x_t")
            nc.sync.dma_start(
                out=x0_t[:],
                in_=x_d[0:ST_ROWS, :].rearrange("(p r) f -> p r f", p=128))
            # csum lands as one 4KB row (sync queue, slotting in right after
            # supertile 0) and is broadcast to all 128 partitions with a
            # rank-1 ones matmul -- saves 1.3us of the DMA-roofline budget
            # vs loading a host-replicated [128, F]
            crow = const_pool.tile([1, F], FP32)
            nc.sync.dma_start(out=crow[:], in_=c_d[:])
            ones = const_pool.tile([1, 128], FP32)
            nc.gpsimd.memset(ones[:], 1.0)
            ident = const_pool.tile([128, 128], FP32)
            make_identity(nc, ident[:])
            csb = const_pool.tile([128, F], FP32)
            for h in range(2):
                cps = tpsum.tile([128, R * 128], FP32, tag="xt_ps")
                nc.tensor.matmul(
                    cps[:], ones[:], crow[:, h * 512:(h + 1) * 512],
                    start=True, stop=True,
                )
                nc.scalar.copy(csb[:, h * 512:(h + 1) * 512], cps[:])

            # w/e ride the SWDGE queue: the sync HWDGE queue is kept for
            # the back-to-back x loads
            wsb_raw = const_pool.tile([128, NCHUNK, L], FP32)
            nc.gpsimd.dma_start(out=wsb_raw[:], in_=w_d[:])
            if dot_f32r:
                # f32r matmul operands must be produced pre-rounded: convert
                # once via ACT (the per-chunk xT copies below do the same)
                wsb = const_pool.tile([128, NCHUNK, L], F32R)
                nc.scalar.copy(wsb[:], wsb_raw[:])
            else:
                wsb = wsb_raw
            esb = const_pool.tile([128, L], FP32)
            nc.gpsimd.dma_start(out=esb[:], in_=e_d[:])

            def body(x0):
                for st in range(N_ST):
                    if st == 0 and x0 is not None:
                        x_t = x0
                    else:
                        x_t = xpool.tile([128, R, F], FP32, tag="x_t")
                        src = x_d[st * ST_ROWS:(st + 1) * ST_ROWS, :].rearrange(
                            "(p r) f -> p r f", p=128)
                        nc.sync.dma_start(out=x_t[:], in_=src)

                    # D^T[i, r*128+j] += sum_f w[i,f] * x[row(4j+r), f]
                    dt_ps = dpsum.tile([L, R * 128], FP32)
                    for c in range(NCHUNK):
                        xt_ps = tpsum.tile([128, R * 128], FP32)
                        for r in range(R):
                            src_ap = x_t[:, r, c * 128:(c + 1) * 128]
                            id_ap = ident[:]
                            if transpose_f32r:
                                src_ap = src_ap.bitcast(F32R)
                                id_ap = id_ap.bitcast(F32R)
                            nc.tensor.matmul(
                                xt_ps[:, r * 128:(r + 1) * 128],
                                src_ap,
                                id_ap,
                                start=True, stop=True,
                                is_transpose=True,
                            )
                        xt_sb = xtpool.tile(
                            [128, R * 128], F32R if dot_f32r else FP32)
                        nc.scalar.copy(xt_sb[:], xt_ps[:])
                        nc.tensor.matmul(
                            dt_ps[:],
                            wsb[:, c, :],
                            xt_sb[:],
                            start=(c == 0), stop=(c == NCHUNK - 1),
                        )

                    dt_sb = spool.tile([L, R * 128], FP32, tag="dt_sb")
                    nc.scalar.copy(dt_sb[:], dt_ps[:])

                    # transpose D^T -> D [128 j, r, i]
                    d_ps = dpsum2.tile([128, R, L], FP32)
                    for r in range(R):
                        nc.tensor.matmul(
                            d_ps[:, r, :],
                            dt_sb[:, r * 128:(r + 1) * 128],
                            ident[:L, :L],
                            start=True, stop=True,
                        )
                    d_sb = spool.tile([128, R, L], FP32, tag="d_sb")
                    nc.scalar.copy(d_sb[:], d_ps[:])

                    # a-recurrence: state=1; state = ((1+d_i)*state) + e_i
                    pd = spool.tile([128, R, L], FP32, tag="pd")
                    nc.vector.tensor_scalar_add(pd[:], d_sb[:], 1.0)
                    sc = spool.tile([128, R, L], FP32, tag="sc")
                    for r in range(R):
                        nc.vector.tensor_tensor_scan(
                            sc[:, r, :], pd[:, r, :], esb[:],
                            1.0, MULT, ADD,
                        )

                    # epilogue in place: x_t = (x_t * a) + csum
                    for r in range(R):
                        nc.vector.scalar_tensor_tensor(
                            x_t[:, r, :], x_t[:, r, :], sc[:, r, L - 1:L],
                            csb[:], MULT, ADD,
                        )
                    dst = o_d[st * ST_ROWS:(st + 1) * ST_ROWS, :].rearrange(
                        "(p r) f -> p r f", p=128)
                    # stores on the SWDGE queue so they never head-of-line
                    # block the next load on the sync HWDGE queue
                    nc.gpsimd.dma_start(out=dst, in_=x_t[:])

            if outer_iters == 1:
                body(x0_t)
            else:
                # chained timing build: every iteration must re-load st 0
                # (the in-place epilogue corrupted it on the previous pass)
                with tc.For_i(0, outer_iters):
                    body(None)
    nc.compile()
    return nc


def _host_prep(Ws, Bs):
    Ws = np.asarray(Ws, dtype=np.float32)
    Bs = np.asarray(Bs, dtype=np.float32)
    # wpack[p, c, i] = Ws[i, c*128 + p]
    wpack = np.ascontiguousarray(
        Ws.reshape(L, NCHUNK, 128).transpose(2, 1, 0))
    csum = np.zeros(F, np.float32)
    e = np.zeros(L, np.float32)
    for i in range(L):
        e[i] = np.float32(csum @ Ws[i])
        csum = (csum + Bs[i]).astype(np.float32)
    erep = np.broadcast_to(e, (128, L)).copy()
    crow = csum.reshape(1, F).copy()
    return wpack, erep, crow


def _get_program(**opts):
    key = tuple(sorted(opts.items()))
    if key not in _PROGRAM_CACHE:
        _PROGRAM_CACHE[key] = _build_program(**opts)
    return _PROGRAM_CACHE[key]


def _in_maps(x, Ws, Bs):
    x = np.asarray(x, dtype=np.float32)
    wpack, erep, crow = _host_prep(Ws, Bs)
    return [
        {
            "x": np.ascontiguousarray(x[k * BS:(k + 1) * BS]),
            "wpack": wpack,
            "erep": erep,
            "crow": crow,
        }
        for k in range(N_CORES)
    ]


def _run(x, Ws, Bs, trace=False, trace_kwargs=None, **opts):
    nc = _get_program(**opts)
    in_maps = _in_maps(x, Ws, Bs)
    res = run_bass_kernel_spmd(
        nc, in_maps, list(range(N_CORES)),
        trace=trace, **(trace_kwargs or {}),
    )
    out = np.concatenate([res.results[k]["out"] for k in range(N_CORES)], axis=0)
    return out, res


def kernel(x, Ws, Bs):
    out, _ = _run(x, Ws, Bs, trace=False)
    return out


# revision 18
# speedup vs baseline: 682.6670x; 668.9977x over previous
"""CrossNetwork (4-layer DCN-v1) Trainium2 Bass kernel.

Math: the reference computes, with x0 = x:
    x_{i+1} = x0 * <x_i, w_i> + b_i + x_i          (i = 0..3)

Every x_i stays of the form  x_i = a_i[row] * x0 + c_i  with
    c_i = sum_{j<i} b_j                (row independent)
    a_{i+1} = a_i * (1 + d_i) + e_i    (per-row scalar recurrence)
    d_i = <x0_row, w_i>                (per-row dots, vs x0 only)
    e_i = <c_i, w_i>                   (scalar constants)
so the full network collapses to:
    out = a_4[:, None] * x0 + (b_0+b_1+b_2+b_3)[None, :]

On-chip per 512-row supertile (layout [128 part, 4 r, 1024 f], row = 4p+r):
  1. PE transposes x chunks -> xT in PSUM
  2. ACT copies xT PSUM->SBUF
  3. PE dot-matmuls  D^T[4, 512] += Wc^T @ xTc   (accumulate over 8 f-chunks)
  4. PE transposes D^T -> D[128, 4r, 4i], ACT copies to SBUF
  5. DVE: 1+D, then tensor_tensor_scan implements the a-recurrence
  6. DVE scalar_tensor_tensor: x_t = (x_t * a) + csum_rep, in place
  7. store x_t (now holding the output) on the SWDGE queue

All 8 supertiles stay resident in SBUF (16KB/partition each), so the 8
input loads pack back-to-back on the sync HWDGE queue while stores
drain on the (otherwise idle) GpSimd SWDGE queue -- DMA, the roofline
resource, never waits on a buffer recycle. Const loads also go out on
the SWDGE queue so the first x load starts immediately.
Sharding: batch dim across 8 cores (4096 rows each), SPMD.
"""

import numpy as np

import concourse.bacc as bacc
import concourse.bass as bass
import concourse.mybir as mybir
import concourse.tile as tile
from concourse.bass_utils import run_bass_kernel_spmd
from concourse.masks import make_identity

N_CORES = 8
B, F, L = 32768, 1024, 4
BS = B // N_CORES          # 4096 rows per core
ST_ROWS = 512              # rows per supertile
N_ST = BS // ST_ROWS       # 8 supertiles per core
R = ST_ROWS // 128         # 4 row-combs per supertile
NCHUNK = F // 128          # 8 feature chunks

FP32 = mybir.dt.float32
ADD = mybir.AluOpType.add
MULT = mybir.AluOpType.mult

_PROGRAM_CACHE = {}


def _build_program(dot_f32r=True, outer_iters=1):
    F32R = mybir.dt.float32r
    nc = bacc.Bacc("TRN2")
    x_d = nc.dram_tensor("x", [BS, F], FP32, kind="ExternalInput")
    w_d = nc.dram_tensor("wpack", [128, NCHUNK, L], FP32, kind="ExternalInput")
    e_d = nc.dram_tensor("erep", [128, L], FP32, kind="ExternalInput")
    c_d = nc.dram_tensor("crow", [1, F], FP32, kind="ExternalInput")
    o_d = nc.dram_tensor("out", [BS, F], FP32, kind="ExternalOutput")

    with tile.TileContext(nc) as tc:
        with (
            tc.tile_pool(name="const", bufs=1) as const_pool,
            tc.tile_pool(name="xin", bufs=N_ST) as xpool,
            tc.tile_pool(name="xtsb", bufs=3) as xtpool,
            tc.tile_pool(name="small", bufs=2) as spool,
            tc.tile_pool(name="xtps", bufs=3, space="PSUM") as tpsum,
            tc.tile_pool(name="dtps", bufs=2, space="PSUM") as dpsum,
            tc.tile_pool(name="dps", bufs=2, space="PSUM") as dpsum2,
        ):
            # supertile 0's load is issued before anything else on the sync
            # queue so the DMA head starts as early as possible
            x0_t = xpool.tile([128, R, F], FP32, tag="x_t")
            nc.sync.dma_start(
                out=x0_t[:],
                in_=x_d[0:ST_ROWS, :].rearrange("(p r) f -> p r f", p=128))
            # csum lands as one 4KB row (sync queue, slotting in right after
            # supertile 0) and is broadcast to all 128 partitions with a
            # rank-1 ones matmul -- saves 1.3us of the DMA-roofline budget
            # vs loading a host-replicated [128, F]
            crow = const_pool.tile([1, F], FP32)
            nc.sync.dma_start(out=crow[:], in_=c_d[:])
            ones = const_pool.tile([1, 128], FP32)
            nc.gpsimd.memset(ones[:], 1.0)
            ident = const_pool.tile([128, 128], FP32)
            make_identity(nc, ident[:])
            csb = const_pool.tile([128, F], FP32)
            for h in range(2):
                cps = tpsum.tile([128, R * 128], FP32, tag="xt_ps")
                nc.tensor.matmul(
                    cps[:], ones[:], crow[:, h * 512:(h + 1) * 512],
                    start=True, stop=True,
                )
                nc.scalar.copy(csb[:, h * 512:(h + 1) * 512], cps[:])

            # w/e ride the SWDGE queue: the sync HWDGE queue is kept for
            # the back-to-back x loads
            wsb_raw = const_pool.tile([128, NCHUNK, L], FP32)
            nc.gpsimd.dma_start(out=wsb_raw[:], in_=w_d[:])
            if dot_f32r:
                # f32r matmul operands must be produced pre-rounded: convert
                # once via ACT (the per-chunk xT copies below do the same)
                wsb = const_pool.tile([128, NCHUNK, L], F32R)
                nc.scalar.copy(wsb[:], wsb_raw[:])
            else:
                wsb = wsb_raw
            esb = const_pool.tile([128, L], FP32)
            nc.gpsimd.dma_start(out=esb[:], in_=e_d[:])

            def body(x0):
                for st in range(N_ST):
                    if st == 0 and x0 is not None:
                        x_t = x0
                    else:
                        x_t = xpool.tile([128, R, F], FP32, tag="x_t")
                        src = x_d[st * ST_ROWS:(st + 1) * ST_ROWS, :].rearrange(
                            "(p r) f -> p r f", p=128)
                        nc.sync.dma_start(out=x_t[:], in_=src)

                    # D^T[i, r*128+j] += sum_f w[i,f] * x[row(4j+r), f]
                    dt_ps = dpsum.tile([L, R * 128], FP32)
                    for c in range(NCHUNK):
                        xt_ps = tpsum.tile([128, R * 128], FP32)
                        for r in range(R):
                            nc.tensor.matmul(
                                xt_ps[:, r * 128:(r + 1) * 128],
                                x_t[:, r, c * 128:(c + 1) * 128],
                                ident[:],
                                start=True, stop=True,
                                is_transpose=True,
                            )
                        xt_sb = xtpool.tile(
                            [128, R * 128], F32R if dot_f32r else FP32)
                        nc.scalar.copy(xt_sb[:], xt_ps[:])
                        nc.tensor.matmul(
                            dt_ps[:],
                            wsb[:, c, :],
                            xt_sb[:],
                            start=(c == 0), stop=(c == NCHUNK - 1),
                        )

                    dt_sb = spool.tile([L, R * 128], FP32, tag="dt_sb")
                    nc.scalar.copy(dt_sb[:], dt_ps[:])

                    # transpose D^T -> D [128 j, r, i]
                    d_ps = dpsum2.tile([128, R, L], FP32)
                    for r in range(R):
                        nc.tensor.matmul(
                            d_ps[:, r, :],
                            dt_sb[:, r * 128:(r + 1) * 128],
                            ident[:L, :L],
                            start=True, stop=True,
                        )
                    d_sb = spool.tile([128, R, L], FP32, tag="d_sb")
                    nc.scalar.copy(d_sb[:], d_ps[:])

                    # a-recurrence: state=1; state = ((1+d_i)*state) + e_i
                    pd = spool.tile([128, R, L], FP32, tag="pd")
                    nc.vector.tensor_scalar_add(pd[:], d_sb[:], 1.0)
                    sc = spool.tile([128, R, L], FP32, tag="sc")
                    for r in range(R):
                        nc.vector.tensor_tensor_scan(
                            sc[:, r, :], pd[:, r, :], esb[:],
                            1.0, MULT, ADD,
                        )

                    # epilogue in place: x_t = (x_t * a) + csum
                    for r in range(R):
                        nc.vector.scalar_tensor_tensor(
                            x_t[:, r, :], x_t[:, r, :], sc[:, r, L - 1:L],
                            csb[:], MULT, ADD,
                        )
                    dst = o_d[st * ST_ROWS:(st + 1) * ST_ROWS, :].rearrange(
                        "(p r) f -> p r f", p=128)
                    # stores on the SWDGE queue so they never head-of-line
                    # block the next load on the sync HWDGE queue
                    nc.gpsimd.dma_start(out=dst, in_=x_t[:])

            if outer_iters == 1:
                body(x0_t)
            else:
                # chained timing build: every iteration must re-load st 0
                # (the in-place epilogue corrupted it on the previous pass)
                with tc.For_i(0, outer_iters):
                    body(None)
    nc.compile()
    return nc


def _host_prep(Ws, Bs):
    Ws = np.asarray(Ws, dtype=np.float32)
    Bs = np.asarray(Bs, dtype=np.float32)
    # wpack[p, c, i] = Ws[i, c*128 + p]
    wpack = np.ascontiguousarray(
        Ws.reshape(L, NCHUNK, 128).transpose(2, 1, 0))
    csum = np.zeros(F, np.float32)
    e = np.zeros(L, np.float32)
    for i in range(L):
        e[i] = np.float32(csum @ Ws[i])
        csum = (csum + Bs[i]).astype(np.float32)
    erep = np.broadcast_to(e, (128, L)).copy()
    crow = csum.reshape(1, F).copy()
    return wpack, erep, crow


def _get_program(**opts):
    key = tuple(sorted(opts.items()))
    if key not in _PROGRAM_CACHE:
        _PROGRAM_CACHE[key] = _build_program(**opts)
    return _PROGRAM_CACHE[key]


def _in_maps(x, Ws, Bs):
    x = np.asarray(x, dtype=np.float32)
    wpack, erep, crow = _host_prep(Ws, Bs)
    return [
        {
            "x": np.ascontiguousarray(x[k * BS:(k + 1) * BS]),
            "wpack": wpack,
            "erep": erep,
            "crow": crow,
        }
        for k in range(N_CORES)
    ]


def _run(x, Ws, Bs, trace=False, trace_kwargs=None, **opts):
    nc = _get_program(**opts)
    in_maps = _in_maps(x, Ws, Bs)
    res = run_bass_kernel_spmd(
        nc, in_maps, list(range(N_CORES)),
        trace=trace, **(trace_kwargs or {}),
    )
    out = np.concatenate([res.results[k]["out"] for k in range(N_CORES)], axis=0)
    return out, res


def kernel(x, Ws, Bs):
    out, _ = _run(x, Ws, Bs, trace=False)
    return out
